# revision 1
# baseline (speedup 1.0000x reference)
"""Trainium2 Bass kernel for nn_HT_56298431316042 (histogram_binning).

Computes  out = relu(image.reshape(32, 16384)) @ vote.reshape(16384, 16384) / 128
         -> reshape (2, 16, 128, 128)

Sharding: column-wise over the 16384 Hough bins -> 2048 bins per core, 8 cores,
no communication. Each core streams its (16384, 2048) slice of the vote matrix
as the matmul moving operand; relu(x)^T chunks are the stationary operand;
accumulation over K=16384 happens in PSUM (fp32).

The vote matrix is binary (0.0/1.0), so casting it to fp16 or fp8e4m3 is
LOSSLESS; only relu(image) rounding is affected by reduced precision:
  - f32 : exact, ~134 MB/core streamed
  - f16 : x rounded to fp16 (rel ~2^-11), ~67 MB/core
  - f8dr: x split into fp8 hi+lo (rel ~2^-8 worst case), ~34 MB/core,
          DoubleRow perf mode (2 contraction rows per cycle)
"""

import numpy as np

import concourse.bass as bass
import concourse.bacc as bacc
import concourse.mybir as mybir
import concourse.tile as tile
from concourse.bass_utils import run_bass_kernel_spmd

MODE = "f8dr"  # one of: f32 | f16 | f8dr

NCORES = 8
B, C, ROWS, COLS, H, W = 2, 16, 128, 128, 128, 128
BC = B * C                      # 32 output rows
K = ROWS * COLS                 # 16384 contraction
NTOT = H * W                    # 16384 output bins
NPC = NTOT // NCORES            # 2048 bins per core
KC = K // 128                   # 128 k-chunks of 128
NT = 512                        # matmul free-dim tile
X_SCALE = {"f32": 1.0, "f16": 1.0, "f8dr": 16.0}
OUT_SCALE = {"f32": 1.0 / COLS, "f16": 1.0 / COLS, "f8dr": 1.0 / (COLS * 16.0)}
VDT = {
    "f32": mybir.dt.float32,
    "f16": mybir.dt.float16,
    "f8dr": mybir.dt.float8e4,
}
# k-chunks per DMA block: keep each dma_start at 2 MiB
GROUP = {"f32": 2, "f16": 4, "f8dr": 8}
VBUFS = {"f32": 4, "f16": 4, "f8dr": 4}

_nc_cache: dict[str, bass.Bass] = {}


def _build(mode: str) -> bass.Bass:
    if mode in _nc_cache:
        return _nc_cache[mode]
    vdt = VDT[mode]
    g = GROUP[mode]
    nb = KC // g
    f32 = mybir.dt.float32

    nc = bacc.Bacc("TRN2", target_bir_lowering=False, debug=False,
                   num_devices=NCORES)
    x_dram = nc.dram_tensor("x", (128, KC * BC), f32, kind="ExternalInput")
    v_dram = nc.dram_tensor("v", (nb, 128, g * NPC + 16), vdt,
                            kind="ExternalInput")
    o_dram = nc.dram_tensor("out", (BC, NPC), f32, kind="ExternalOutput")

    vbufs = VBUFS[mode]
    with tile.TileContext(nc) as tc:
        with tc.tile_pool(name="xp", bufs=1) as xp, \
             tc.tile_pool(name="vp", bufs=1) as vp, \
             tc.tile_pool(name="pp", bufs=1, space="PSUM") as pp, \
             tc.tile_pool(name="pt", bufs=1, space="PSUM") as pt, \
             tc.tile_pool(name="gs", bufs=nb) as gate_pool, \
             tc.tile_pool(name="op", bufs=1) as op:

            # --- x preparation: load, relu(+scale), cast/split ---
            x_raw = xp.tile([128, KC * BC], f32)
            nc.scalar.dma_start(out=x_raw[:], in_=x_dram.ap())

            relu = mybir.ActivationFunctionType.Relu
            if mode == "f32":
                x_use = xp.tile([128, KC * BC], f32)
                nc.scalar.activation(x_use[:], x_raw[:], relu)
                passes = [x_use]
            elif mode == "f16":
                x_use = xp.tile([128, KC * BC], mybir.dt.float16)
                nc.scalar.activation(x_use[:], x_raw[:], relu)
                passes = [x_use]
            else:  # f8dr: hi/lo split of relu(x)*16
                x_rel = xp.tile([128, KC * BC], f32)
                nc.scalar.activation(x_rel[:], x_raw[:], relu,
                                     scale=X_SCALE[mode])
                x_hi = xp.tile([128, KC * BC], vdt)
                nc.vector.tensor_copy(out=x_hi[:], in_=x_rel[:])
                x_hi32 = xp.tile([128, KC * BC], f32)
                nc.vector.tensor_copy(out=x_hi32[:], in_=x_hi[:])
                resid = xp.tile([128, KC * BC], f32)
                nc.vector.tensor_sub(resid[:], x_rel[:], x_hi32[:])
                x_lo = xp.tile([128, KC * BC], vdt)
                nc.vector.tensor_copy(out=x_lo[:], in_=resid[:])
                passes = [x_hi, x_lo]

            psum = pp.tile([BC, NPC], f32)

            # Walrus allows only ONE sem-wait per DMA instruction, but a
            # v-block DMA into a reused pool slot needs two: WAR on the
            # stale tile's PE readers + WAW on the slot's previous DMA
            # (Tile doesn't collapse waits transitively). Fix:
            #  - every block ends with a tiny "token" matmul into a
            #    dedicated PSUM bank (last PE op touching the block's tile)
            #  - before reusing a slot, ACT copies that token from PSUM
            #    into the stale tile: this gate carries the single PE wait
            #    and its write WAW-orders it before the real DMA on ACT
            #  - the real DMA (also issued from ACT) then carries only the
            #    DMA-lane WAW wait: every instruction has <= 1 sem wait.
            vtiles: list = []
            tok = []
            vts = []
            for j in range(vbufs):
                tok_t = pt.tile([1, 16], f32, tag=f"tok{j}")
                tok.append(tok_t)
                vt_t = vp.tile([128, g * NPC + 16], vdt, tag=f"vt{j}")
                vts.append(vt_t)
            def gate(b):
                if b >= vbufs:
                    stale = vtiles[b - vbufs]
                    # absorb the stale slot's DMA-lane tick into ACT
                    # program order (1 wait: old DMA lane); fresh scratch
                    # slot every time so no WAW self-wait accumulates
                    pg_t = gate_pool.tile([1, 16], f32, tag="pg")
                    nc.scalar.copy(pg_t[:], stale[0:1, 16:32])
                    # carry the PE release (1 wait: PE >= token-mm), and
                    # WAW-order the real DMA behind us on ACT via the junk
                    # pad columns (PE never reads those)
                    nc.scalar.copy(stale[0:1, g * NPC:g * NPC + 16],
                                   tok[(b - vbufs) % vbufs][:])

            def token_mm(b, vt2d, lhs_src):
                nc.tensor.matmul(tok[b % vbufs][:], lhsT=lhs_src[:, 0:1],
                                 rhs=vt2d[:, 0:16], start=True, stop=True)

            # --- main loop: stream V blocks, accumulate matmuls ---
            if mode == "f8dr":
                dr = mybir.MatmulPerfMode.DoubleRow
                gg_per_block = g // 2
                for b in range(nb):
                    gate(b)
                    vt2d = vts[b % vbufs]
                    vtiles.append(vt2d)
                    nc.scalar.dma_start(out=vt2d[:], in_=v_dram.ap()[b])
                    vt = vt2d[:, 0:g * NPC].rearrange(
                        "p (gg j n) -> p gg j n", gg=gg_per_block, j=2)
                    for gg in range(gg_per_block):
                        cc = b * gg_per_block + gg   # 0..63 double-chunks
                        first = cc == 0
                        last = cc == KC // 2 - 1
                        for n in range(NPC // NT):
                            rhs = vt[:, gg, :, n * NT:(n + 1) * NT]
                            for ip, xpass in enumerate(passes):
                                lhsT = xpass[:, 2 * cc * BC:(2 * cc + 2) * BC]
                                lhsT = lhsT.rearrange(
                                    "p (j m) -> p j m", j=2)
                                nc.tensor.matmul(
                                    psum[:, n * NT:(n + 1) * NT],
                                    lhsT=lhsT, rhs=rhs,
                                    start=(first and ip == 0),
                                    stop=(last and ip == len(passes) - 1),
                                    perf_mode=dr)
                    token_mm(b, vt2d, passes[0])
            else:
                for b in range(nb):
                    gate(b)
                    vt = vts[b % vbufs]
                    vtiles.append(vt)
                    nc.scalar.dma_start(out=vt[:], in_=v_dram.ap()[b])
                    for i in range(g):
                        c = b * g + i
                        lhsT = passes[0][:, c * BC:(c + 1) * BC]
                        for n in range(NPC // NT):
                            nc.tensor.matmul(
                                psum[:, n * NT:(n + 1) * NT],
                                lhsT=lhsT,
                                rhs=vt[:, i * NPC + n * NT:
                                       i * NPC + (n + 1) * NT],
                                start=(c == 0), stop=(c == KC - 1))
                    token_mm(b, vt, passes[0])

            # --- epilogue: flush the last blocks' DMA-lane ticks into ACT
            # so the kernel-tail Drain doesn't exceed its wait capacity ---
            for bb in range(max(0, nb - vbufs), nb):
                fl_t = gate_pool.tile([1, 16], f32, tag="pg")
                nc.scalar.copy(fl_t[:], vtiles[bb][0:1, 16:32])

            # --- epilogue: scale + store ---
            out_t = op.tile([BC, NPC], f32)
            nc.scalar.mul(out_t[:], psum[:], OUT_SCALE[mode])
            nc.scalar.dma_start(out=o_dram.ap(), in_=out_t[:])

    nc.finalize()
    _nc_cache[mode] = nc
    return nc


def _prep_inputs(image: np.ndarray, vote_index: np.ndarray, mode: str):
    np_vdt = mybir.dt.np(VDT[mode])
    g = GROUP[mode]
    nb = KC // g

    # x arranged (128, KC*BC): [p, c*32+m] = image_flat[m, c*128+p] * X_SCALE
    x2 = np.ascontiguousarray(image.reshape(BC, K), dtype=np.float32)
    x_arr = np.ascontiguousarray(
        x2.reshape(BC, KC, 128).transpose(2, 1, 0)).reshape(128, KC * BC)

    # v arranged per core: (nb, 128, g*NPC): [b, p, g'*NPC+j] =
    #   V[(b*g+g')*128 + p, core*NPC + j]
    v2 = vote_index.reshape(K, NTOT)
    if np_vdt != np.float32:
        v2 = v2.astype(np_vdt)  # binary 0/1 -> lossless
    # reshape [b, g', p, core, j] -> transpose to [core, b, p, g', j]
    v5 = v2.reshape(nb, g, 128, NCORES, NPC).transpose(3, 0, 2, 1, 4)
    in_maps = []
    for i in range(NCORES):
        vi = np.zeros((nb, 128, g * NPC + 16), dtype=np_vdt)
        vi[:, :, :g * NPC] = v5[i].reshape(nb, 128, g * NPC)
        in_maps.append({"x": x_arr, "v": vi})
    return in_maps


def _run(image, vote_index, mode=None, **run_kwargs):
    mode = mode or MODE
    nc = _build(mode)
    in_maps = _prep_inputs(np.asarray(image), np.asarray(vote_index), mode)
    res = run_bass_kernel_spmd(nc, in_maps, core_ids=list(range(NCORES)),
                               **run_kwargs)
    out = np.concatenate([r["out"] for r in res.results], axis=1)
    return out.reshape(B, C, H, W).astype(np.float32), res


def kernel(image: np.ndarray, vote_index: np.ndarray) -> np.ndarray:
    out, _ = _run(image, vote_index)
    return out



# revision 18
# speedup vs baseline: 2.3518x; 2.3518x over previous
"""Trainium2 Bass kernel for nn_HT_56298431316042 (histogram_binning).

Computes  out = relu(image.reshape(32, 16384)) @ vote.reshape(16384, 16384) / 128
         -> reshape (2, 16, 128, 128)

Sharding: column-wise over the 16384 Hough bins -> 2048 bins per core, 8 cores,
no communication. Each core streams its (16384, 2048) slice of the vote matrix
as the matmul moving operand; relu(x)^T chunks are the stationary operand;
accumulation over K=16384 happens in PSUM (fp32).

The vote matrix is binary (0.0/1.0), so casting it to fp16 or fp8e4m3 is
LOSSLESS; only relu(image) rounding is affected by reduced precision:
  - f32 : exact, ~134 MB/core streamed
  - f16 : x rounded to fp16 (rel ~2^-11), ~67 MB/core
  - f8dr: x split into fp8 hi+lo (rel ~2^-8 worst case), ~34 MB/core,
          DoubleRow perf mode (2 contraction rows per cycle)
"""

import numpy as np

import concourse.bass as bass
import concourse.bacc as bacc
import concourse.mybir as mybir
import concourse.tile as tile
from concourse.bass_utils import run_bass_kernel_spmd

MODE = "hyb"  # one of: f32 | f16 | f8dr | f8s | hyb

NCORES = 8
B, C, ROWS, COLS, H, W = 2, 16, 128, 128, 128, 128
BC = B * C                      # 32 output rows
K = ROWS * COLS                 # 16384 contraction
NTOT = H * W                    # 16384 output bins
NPC = NTOT // NCORES            # 2048 bins per core
KC = K // 128                   # 128 k-chunks of 128
NT = 512                        # matmul free-dim tile
X_SCALE = {"f32": 1.0, "f16": 1.0, "f8dr": 16.0, "f8s": 16.0}
OUT_SCALE = {"f32": 1.0 / COLS, "f16": 1.0 / COLS,
             "f8dr": 1.0 / (COLS * 16.0), "f8s": 1.0 / (COLS * 16.0)}
VDT = {
    "f32": mybir.dt.float32,
    "f16": mybir.dt.float16,
    "f8dr": mybir.dt.float8e4,
    "f8s": mybir.dt.float8e4,
}
# k-chunks per DMA block: keep each dma_start at 2 MiB (1 MiB for f8s)
GROUP = {"f32": 2, "f16": 4, "f8dr": 8, "f8s": 4}
VBUFS = {"f32": 4, "f16": 4, "f8dr": 4, "f8s": 4}

_nc_cache: dict[str, bass.Bass] = {}

# ---- hybrid-mode constants ----------------------------------------------
# 64 DoubleRow chunk-pairs (128 k-chunks of 128 rows) from two producers,
# balanced so DMA_ENGINES and DVE finish together:
#   - dense pairs arrive as fp8 via DMA (360 B/ns shared bus)
#   - packed pairs arrive as 1 bit/elem uint16 words; ONE DVE tensor_scalar
#     per pair extracts two bit-planes at once:
#       (U & ((1<<s)|(1<<(s+8)))) << (6-s)   [s=7: >> 1]
#     leaving 0x40 in the lo byte (plane j=0) and 0x4000's hi byte (plane
#     j=1); the uint16 buffer bitcast to fp8 reads 2.0/0.0 in exactly the
#     byte-interleaved layout DoubleRow wants (j stride 1, n stride 2).
# Scale bookkeeping: dense chunks use x*16 (V=1.0), packed use x*8 (V=2.0),
# so one PSUM accumulator holds 16*(x@V) and OUT_SCALE=1/(128*16) for both.
HYB_DPAIRS = 13                  # dense chunk-pairs, one per DMA block
HYB_PPAIRS = 64 - HYB_DPAIRS     # packed pairs (8 per uint16 word group)
HYB_GROUPS = (HYB_PPAIRS + 7) // 8
HYB_DVE_NS = 594                 # est ns per packed pair on DVE
HYB_XFER_D = 1490                # est ns per dense pair DMA
HYB_XFER_G = 1456                # est ns per group / x DMA

def _build(mode: str) -> bass.Bass:
    if mode in _nc_cache:
        return _nc_cache[mode]
    if mode == "hyb":
        nc = _build_hyb()
        _nc_cache[mode] = nc
        return nc
    vdt = VDT[mode]
    g = GROUP[mode]
    nb = KC // g
    f32 = mybir.dt.float32

    nc = bacc.Bacc("TRN2", target_bir_lowering=False, debug=False,
                   num_devices=NCORES)
    xdt = vdt if mode == "f8s" else f32
    x_dram = nc.dram_tensor("x", (128, KC * BC), xdt, kind="ExternalInput")
    v_dram = nc.dram_tensor("v", (nb, 128, g * NPC + 16), vdt,
                            kind="ExternalInput")
    o_dram = nc.dram_tensor("out", (BC, NPC), f32, kind="ExternalOutput")

    vbufs = VBUFS[mode]
    with tile.TileContext(nc) as tc:
        with tc.tile_pool(name="xp", bufs=1) as xp, \
             tc.tile_pool(name="vp", bufs=1) as vp, \
             tc.tile_pool(name="pp", bufs=1, space="PSUM") as pp, \
             tc.tile_pool(name="pt", bufs=1, space="PSUM") as pt, \
             tc.tile_pool(name="gs", bufs=nb) as gate_pool, \
             tc.tile_pool(name="op", bufs=1) as op:

            # --- x preparation: load, relu(+scale), cast/split ---
            x_raw = xp.tile([128, KC * BC], xdt)
            nc.scalar.dma_start(out=x_raw[:], in_=x_dram.ap())

            relu = mybir.ActivationFunctionType.Relu
            if mode == "f8s":
                # host sent fp8e4m3(16*x); relu on DVE keeps the ACT queue
                # free to issue the V-block DMA stream without stalls
                x_use = xp.tile([128, KC * BC], vdt)
                nc.vector.tensor_relu(x_use[:], x_raw[:])
                passes = [x_use]
            elif mode == "f32":
                x_use = xp.tile([128, KC * BC], f32)
                nc.scalar.activation(x_use[:], x_raw[:], relu)
                passes = [x_use]
            elif mode == "f16":
                x_use = xp.tile([128, KC * BC], mybir.dt.float16)
                nc.scalar.activation(x_use[:], x_raw[:], relu)
                passes = [x_use]
            else:  # f8dr: hi/lo split of relu(x)*16
                x_rel = xp.tile([128, KC * BC], f32)
                nc.scalar.activation(x_rel[:], x_raw[:], relu,
                                     scale=X_SCALE[mode])
                x_hi = xp.tile([128, KC * BC], vdt)
                nc.vector.tensor_copy(out=x_hi[:], in_=x_rel[:])
                x_hi32 = xp.tile([128, KC * BC], f32)
                nc.vector.tensor_copy(out=x_hi32[:], in_=x_hi[:])
                resid = xp.tile([128, KC * BC], f32)
                nc.vector.tensor_sub(resid[:], x_rel[:], x_hi32[:])
                x_lo = xp.tile([128, KC * BC], vdt)
                nc.vector.tensor_copy(out=x_lo[:], in_=resid[:])
                passes = [x_hi, x_lo]

            psum = pp.tile([BC, NPC], f32)

            # Walrus allows only ONE sem-wait per DMA instruction, but a
            # v-block DMA into a reused pool slot needs two: WAR on the
            # stale tile's PE readers + WAW on the slot's previous DMA
            # (Tile doesn't collapse waits transitively). Fix:
            #  - every block ends with a tiny "token" matmul into a
            #    dedicated PSUM bank (last PE op touching the block's tile)
            #  - before reusing a slot, ACT copies that token from PSUM
            #    into the stale tile: this gate carries the single PE wait
            #    and its write WAW-orders it before the real DMA on ACT
            #  - the real DMA (also issued from ACT) then carries only the
            #    DMA-lane WAW wait: every instruction has <= 1 sem wait.
            vtiles: list = []
            tok = []
            vts = []
            for j in range(vbufs):
                tok_t = pt.tile([1, 16], f32, tag=f"tok{j}")
                tok.append(tok_t)
                vt_t = vp.tile([128, g * NPC + 16], vdt, tag=f"vt{j}")
                vts.append(vt_t)
            def gate(b):
                if b >= vbufs:
                    stale = vtiles[b - vbufs]
                    # absorb the stale slot's DMA-lane tick into ACT
                    # program order (1 wait: old DMA lane); fresh scratch
                    # slot every time so no WAW self-wait accumulates
                    pg_t = gate_pool.tile([1, 16], f32, tag="pg")
                    nc.scalar.copy(pg_t[:], stale[0:1, 16:32])
                    # carry the PE release (1 wait: PE >= token-mm), and
                    # WAW-order the real DMA behind us on ACT via the junk
                    # pad columns (PE never reads those)
                    nc.scalar.copy(stale[0:1, g * NPC:g * NPC + 16],
                                   tok[(b - vbufs) % vbufs][:])

            def token_mm(b, vt2d, lhs_src):
                nc.tensor.matmul(tok[b % vbufs][:], lhsT=lhs_src[:, 0:1],
                                 rhs=vt2d[:, 0:16], start=True, stop=True)

            # --- main loop: stream V blocks, accumulate matmuls ---
            if mode in ("f8dr", "f8s"):
                dr = mybir.MatmulPerfMode.DoubleRow
                gg_per_block = g // 2
                for b in range(nb):
                    gate(b)
                    vt2d = vts[b % vbufs]
                    vtiles.append(vt2d)
                    nc.scalar.dma_start(out=vt2d[:], in_=v_dram.ap()[b])
                    vt = vt2d[:, 0:g * NPC].rearrange(
                        "p (gg j n) -> p gg j n", gg=gg_per_block, j=2)
                    for gg in range(gg_per_block):
                        cc = b * gg_per_block + gg   # 0..63 double-chunks
                        first = cc == 0
                        last = cc == KC // 2 - 1
                        for n in range(NPC // NT):
                            rhs = vt[:, gg, :, n * NT:(n + 1) * NT]
                            for ip, xpass in enumerate(passes):
                                lhsT = xpass[:, 2 * cc * BC:(2 * cc + 2) * BC]
                                lhsT = lhsT.rearrange(
                                    "p (j m) -> p j m", j=2)
                                nc.tensor.matmul(
                                    psum[:, n * NT:(n + 1) * NT],
                                    lhsT=lhsT, rhs=rhs,
                                    start=(first and ip == 0),
                                    stop=(last and ip == len(passes) - 1),
                                    perf_mode=dr)
                    token_mm(b, vt2d, passes[0])
            else:
                for b in range(nb):
                    gate(b)
                    vt = vts[b % vbufs]
                    vtiles.append(vt)
                    nc.scalar.dma_start(out=vt[:], in_=v_dram.ap()[b])
                    for i in range(g):
                        c = b * g + i
                        lhsT = passes[0][:, c * BC:(c + 1) * BC]
                        for n in range(NPC // NT):
                            nc.tensor.matmul(
                                psum[:, n * NT:(n + 1) * NT],
                                lhsT=lhsT,
                                rhs=vt[:, i * NPC + n * NT:
                                       i * NPC + (n + 1) * NT],
                                start=(c == 0), stop=(c == KC - 1))
                    token_mm(b, vt, passes[0])

            # --- epilogue: flush the last blocks' DMA-lane ticks into ACT
            # so the kernel-tail Drain doesn't exceed its wait capacity ---
            for bb in range(max(0, nb - vbufs), nb):
                fl_t = gate_pool.tile([1, 16], f32, tag="pg")
                nc.scalar.copy(fl_t[:], vtiles[bb][0:1, 16:32])

            # --- epilogue: scale + store ---
            out_t = op.tile([BC, NPC], f32)
            nc.scalar.mul(out_t[:], psum[:], OUT_SCALE[mode])
            nc.scalar.dma_start(out=o_dram.ap(), in_=out_t[:])

    nc.finalize()
    _nc_cache[mode] = nc
    return nc


def _build_hyb() -> bass.Bass:
    f32 = mybir.dt.float32
    f8 = mybir.dt.float8e4
    u16 = mybir.dt.uint16
    alu = mybir.AluOpType
    dr = mybir.MatmulPerfMode.DoubleRow
    nb = HYB_DPAIRS                  # one dense pair per DMA block
    vbufs = 4
    ebufs = 6

    nc = bacc.Bacc("TRN2", target_bir_lowering=False, debug=False,
                   num_devices=NCORES)
    x_dram = nc.dram_tensor("x", (128, KC * BC), f8, kind="ExternalInput")
    v_dram = nc.dram_tensor("v", (nb, 128, 2 * NPC + 16), f8,
                            kind="ExternalInput")
    u_dram = nc.dram_tensor("u", (HYB_GROUPS, 128, NPC), u16,
                            kind="ExternalInput")
    o_dram = nc.dram_tensor("out", (BC, NPC), f32, kind="ExternalOutput")

    # ---- static schedule: estimate producer completion times -------------
    # SP queue issues all ungated DMAs (x, groups, dense blocks < vbufs),
    # interleaved so DVE gets its first group immediately; gated dense
    # blocks issue from ACT inline with their matmuls.
    sp_issue = [("g", 0), ("x",), ("g", 1), ("d", 0), ("g", 2), ("d", 1),
                ("g", 3), ("d", 2), ("g", 4), ("d", 3), ("g", 5), ("g", 6)]
    sp_issue = [it for it in sp_issue
                if not (it[0] == "g" and it[1] >= HYB_GROUPS)
                and not (it[0] == "d" and it[1] >= min(vbufs, nb))]
    t, g_ready, d_ready = 0.0, {}, {}
    for it in sp_issue:
        t += HYB_XFER_D if it[0] == "d" else HYB_XFER_G
        if it[0] == "g":
            g_ready[it[1]] = t
        elif it[0] == "d":
            d_ready[it[1]] = t
    for b in range(min(vbufs, nb), nb):
        t += HYB_XFER_D
        d_ready[b] = t
    dve_est, tt = {}, 0.0
    for l in range(HYB_PPAIRS):
        tt = max(tt, g_ready[l // 8]) + HYB_DVE_NS
        dve_est[l] = tt
    items = ([("d", b, d_ready[b]) for b in range(nb)]
             + [("v", l, dve_est[l]) for l in range(HYB_PPAIRS)])
    items.sort(key=lambda it: it[2])

    with tile.TileContext(nc) as tc:
        with tc.tile_pool(name="xp", bufs=1) as xp, \
             tc.tile_pool(name="vp", bufs=1) as vp, \
             tc.tile_pool(name="gp", bufs=1) as gp, \
             tc.tile_pool(name="ep", bufs=1) as ep, \
             tc.tile_pool(name="pp", bufs=1, space="PSUM") as pp, \
             tc.tile_pool(name="pt", bufs=1, space="PSUM") as pt, \
             tc.tile_pool(name="gs", bufs=nb + 2) as gate_pool, \
             tc.tile_pool(name="op", bufs=1) as op:

            gtiles = [gp.tile([128, NPC], u16, tag=f"g{t_}",
                              name=f"gt{t_}") for t_ in range(HYB_GROUPS)]
            x_raw = xp.tile([128, KC * BC], f8)
            x_use = xp.tile([128, KC * BC], f8)

            psum = pp.tile([BC, NPC], f32)
            tok = [pt.tile([1, 16], f32, tag=f"tok{j}", name=f"tok{j}")
                   for j in range(vbufs)]
            vts = [vp.tile([128, 2 * NPC + 16], f8, tag=f"vt{j}",
                           name=f"vt{j}") for j in range(vbufs)]
            etiles = [ep.tile([128, NPC], u16, tag=f"e{j}",
                              name=f"et{j}") for j in range(ebufs)]
            vtiles: list = []

            def gate(b):
                if b >= vbufs:
                    stale = vtiles[b - vbufs]
                    pg_t = gate_pool.tile([1, 16], f32, tag="pg")
                    nc.scalar.copy(pg_t[:], stale[0:1, 16:32])
                    nc.scalar.copy(stale[0:1, 2 * NPC:2 * NPC + 16],
                                   tok[(b - vbufs) % vbufs][:])

            def issue_dense(b, eng):
                gate(b)
                vt2d = vts[b % vbufs]
                vtiles.append(vt2d)
                eng.dma_start(out=vt2d[:], in_=v_dram.ap()[b])

            # ---- SP queue: all ungated input DMAs ------------------------
            for it in sp_issue:
                if it[0] == "g":
                    nc.sync.dma_start(out=gtiles[it[1]][:],
                                      in_=u_dram.ap()[it[1]])
                elif it[0] == "x":
                    nc.sync.dma_start(out=x_raw[:], in_=x_dram.ap())
                else:
                    issue_dense(it[1], nc.sync)

            # ---- ACT: relu (x arrives early on SP) -----------------------
            nc.scalar.activation(x_use[:], x_raw[:],
                                 mybir.ActivationFunctionType.Relu)

            def pair_matmuls(cc, rhs_jn, first, last):
                lhsT = x_use[:, 2 * cc * BC:(2 * cc + 2) * BC].rearrange(
                    "p (j m) -> p j m", j=2)
                for n in range(NPC // NT):
                    nc.tensor.matmul(
                        psum[:, n * NT:(n + 1) * NT], lhsT=lhsT,
                        rhs=rhs_jn[:, :, n * NT:(n + 1) * NT],
                        start=first, stop=last, perf_mode=dr)

            # ---- matmuls (+ inline gated DMA / expansion) in est order ---
            ecnt = 0
            for idx, it in enumerate(items):
                first = idx == 0
                last = idx == len(items) - 1
                if it[0] == "d":
                    b = it[1]
                    if b >= min(vbufs, nb):
                        issue_dense(b, nc.scalar)  # gate + DMA on ACT
                    vt = vtiles[b][:, 0:2 * NPC].rearrange(
                        "p (j n) -> p j n", j=2)
                    pair_matmuls(b, vt, first, last)
                    nc.tensor.matmul(tok[b % vbufs][:], lhsT=x_use[:, 0:1],
                                     rhs=vtiles[b][:, 0:16],
                                     start=True, stop=True)
                else:
                    l = it[1]
                    grp, s = l // 8, l % 8
                    slot = etiles[ecnt % ebufs]
                    ecnt += 1
                    if s <= 6:
                        op1, s2 = alu.logical_shift_left, 6 - s
                    else:
                        op1, s2 = alu.logical_shift_right, 1
                    nc.vector.tensor_scalar(
                        out=slot[:], in0=gtiles[grp][:],
                        scalar1=(1 << s) | (1 << (s + 8)), scalar2=s2,
                        op0=alu.bitwise_and, op1=op1)
                    cc = HYB_DPAIRS + l
                    rhs = slot[:].bitcast(f8).rearrange(
                        "p (n j) -> p j n", j=2)
                    pair_matmuls(cc, rhs, first, last)

            # ---- epilogue: drain dense DMA-lane ticks, scale, store ------
            for bb in range(max(0, nb - vbufs), nb):
                fl_t = gate_pool.tile([1, 16], f32, tag="pg")
                nc.scalar.copy(fl_t[:], vtiles[bb][0:1, 16:32])
            out_t = op.tile([BC, NPC], f32)
            nc.scalar.mul(out_t[:], psum[:], 1.0 / (COLS * 16.0))
            nc.scalar.dma_start(out=o_dram.ap(), in_=out_t[:])

    nc.finalize()
    return nc


def _prep_inputs(image: np.ndarray, vote_index: np.ndarray, mode: str):
    if mode == "hyb":
        return _prep_inputs_hyb(image, vote_index)
    np_vdt = mybir.dt.np(VDT[mode])
    g = GROUP[mode]
    nb = KC // g

    # x arranged (128, KC*BC): [p, c*32+m] = image_flat[m, c*128+p] * X_SCALE
    x2 = np.ascontiguousarray(image.reshape(BC, K), dtype=np.float32)
    x_arr = np.ascontiguousarray(
        x2.reshape(BC, KC, 128).transpose(2, 1, 0)).reshape(128, KC * BC)
    if mode == "f8s":
        # pre-relu fp8 cast of 16*x: fp8 rounding preserves sign, so
        # relu(fp8(16x)) == fp8(16*relu(x)) — relu itself stays on device
        x_arr = (x_arr * X_SCALE[mode]).astype(np_vdt)

    # v arranged per core: (nb, 128, g*NPC): [b, p, g'*NPC+j] =
    #   V[(b*g+g')*128 + p, core*NPC + j]
    v2 = vote_index.reshape(K, NTOT)
    if np_vdt != np.float32:
        v2 = v2.astype(np_vdt)  # binary 0/1 -> lossless
    # reshape [b, g', p, core, j] -> transpose to [core, b, p, g', j]
    v5 = v2.reshape(nb, g, 128, NCORES, NPC).transpose(3, 0, 2, 1, 4)
    in_maps = []
    for i in range(NCORES):
        vi = np.zeros((nb, 128, g * NPC + 16), dtype=np_vdt)
        vi[:, :, :g * NPC] = v5[i].reshape(nb, 128, g * NPC)
        in_maps.append({"x": x_arr, "v": vi})
    return in_maps


def _prep_inputs_hyb(image: np.ndarray, vote_index: np.ndarray):
    np_f8 = mybir.dt.np(mybir.dt.float8e4)
    nb = HYB_DPAIRS
    dchunks = 2 * HYB_DPAIRS

    # x arranged (128, KC*BC): [p, c*32+m] = image_flat[m, c*128+p] * s(c)
    # s = 16 for dense chunks (V encoded as 1.0), 8 for packed (V reads 2.0)
    x2 = np.ascontiguousarray(image.reshape(BC, K), dtype=np.float32)
    x_arr = np.ascontiguousarray(
        x2.reshape(BC, KC, 128).transpose(2, 1, 0)).reshape(128, KC * BC)
    scales = np.where(np.arange(KC) < dchunks, 16.0, 8.0).astype(np.float32)
    x_arr = (x_arr.reshape(128, KC, BC) * scales[None, :, None]
             ).reshape(128, KC * BC).astype(np_f8)

    v2 = vote_index.reshape(K, NTOT)
    # dense file: (nb, 128, 2*NPC+16); block b col = j*NPC + n, chunk 2b+j
    vd = v2[:dchunks * 128].reshape(nb, 2, 128, NCORES, NPC)
    vd = vd.transpose(3, 0, 2, 1, 4)  # (core, b, p, j, n)
    # packed file: (groups, 128, NPC) uint16; packed pair l = chunks
    # (dchunks+2l, dchunks+2l+1) -> group l//8, bits (l%8, l%8+8)
    pchunks = K // 128 - dchunks
    vp = v2[dchunks * 128:].reshape(pchunks, 128, NCORES, NPC)
    u_all = np.zeros((HYB_GROUPS, 128, NCORES, NPC), dtype=np.uint16)
    for q in range(pchunks):
        l, j = q // 2, q % 2
        bit = (l % 8) + 8 * j
        u_all[l // 8] |= vp[q].astype(np.uint16) << np.uint16(bit)

    in_maps = []
    for i in range(NCORES):
        vi = np.zeros((nb, 128, 2 * NPC + 16), dtype=np_f8)
        vi[:, :, :2 * NPC] = vd[i].reshape(nb, 128, 2 * NPC).astype(np_f8)
        ui = np.ascontiguousarray(u_all[:, :, i, :])
        in_maps.append({"x": x_arr, "v": vi, "u": ui})
    return in_maps


def _run(image, vote_index, mode=None, **run_kwargs):
    mode = mode or MODE
    nc = _build(mode)
    in_maps = _prep_inputs(np.asarray(image), np.asarray(vote_index), mode)
    res = run_bass_kernel_spmd(nc, in_maps, core_ids=list(range(NCORES)),
                               **run_kwargs)
    out = np.concatenate([r["out"] for r in res.results], axis=1)
    return out.reshape(B, C, H, W).astype(np.float32), res


def kernel(image: np.ndarray, vote_index: np.ndarray) -> np.ndarray:
    out, _ = _run(image, vote_index)
    return out



# revision 23
# speedup vs baseline: 2.7491x; 1.1689x over previous
"""Trainium2 Bass kernel for nn_HT_56298431316042 (histogram_binning).

Computes  out = relu(image.reshape(32, 16384)) @ vote.reshape(16384, 16384) / 128
         -> reshape (2, 16, 128, 128)

Sharding: column-wise over the 16384 Hough bins -> 2048 bins per core, 8 cores,
no communication. Each core streams its (16384, 2048) slice of the vote matrix
as the matmul moving operand; relu(x)^T chunks are the stationary operand;
accumulation over K=16384 happens in PSUM (fp32).

The vote matrix is binary (0.0/1.0), so casting it to fp16 or fp8e4m3 is
LOSSLESS; only relu(image) rounding is affected by reduced precision:
  - f32 : exact, ~134 MB/core streamed
  - f16 : x rounded to fp16 (rel ~2^-11), ~67 MB/core
  - f8dr: x split into fp8 hi+lo (rel ~2^-8 worst case), ~34 MB/core,
          DoubleRow perf mode (2 contraction rows per cycle)
"""

import numpy as np

import concourse.bass as bass
import concourse.bacc as bacc
import concourse.mybir as mybir
import concourse.tile as tile
from concourse.bass_utils import run_bass_kernel_spmd

MODE = "hyb"  # one of: f32 | f16 | f8dr | f8s | hyb

NCORES = 8
B, C, ROWS, COLS, H, W = 2, 16, 128, 128, 128, 128
BC = B * C                      # 32 output rows
K = ROWS * COLS                 # 16384 contraction
NTOT = H * W                    # 16384 output bins
NPC = NTOT // NCORES            # 2048 bins per core
KC = K // 128                   # 128 k-chunks of 128
NT = 512                        # matmul free-dim tile
X_SCALE = {"f32": 1.0, "f16": 1.0, "f8dr": 16.0, "f8s": 16.0}
OUT_SCALE = {"f32": 1.0 / COLS, "f16": 1.0 / COLS,
             "f8dr": 1.0 / (COLS * 16.0), "f8s": 1.0 / (COLS * 16.0)}
VDT = {
    "f32": mybir.dt.float32,
    "f16": mybir.dt.float16,
    "f8dr": mybir.dt.float8e4,
    "f8s": mybir.dt.float8e4,
}
# k-chunks per DMA block: keep each dma_start at 2 MiB (1 MiB for f8s)
GROUP = {"f32": 2, "f16": 4, "f8dr": 8, "f8s": 4}
VBUFS = {"f32": 4, "f16": 4, "f8dr": 4, "f8s": 4}

_nc_cache: dict[str, bass.Bass] = {}

# ---- hybrid-mode constants ----------------------------------------------
# 64 DoubleRow chunk-pairs (128 k-chunks of 128 rows) from two producers,
# balanced so DMA_ENGINES and DVE finish together:
#   - dense pairs arrive as fp8 via DMA (360 B/ns shared bus)
#   - packed pairs arrive as 1 bit/elem uint16 words; ONE DVE tensor_scalar
#     per pair extracts two bit-planes at once:
#       (U & ((1<<s)|(1<<(s+8)))) << (6-s)   [s=7: >> 1]
#     leaving 0x40 in the lo byte (plane j=0) and 0x4000's hi byte (plane
#     j=1); the uint16 buffer bitcast to fp8 reads 2.0/0.0 in exactly the
#     byte-interleaved layout DoubleRow wants (j stride 1, n stride 2).
# Scale bookkeeping: dense chunks use x*16 (V=1.0), packed use x*8 (V=2.0),
# so one PSUM accumulator holds 16*(x@V) and OUT_SCALE=1/(128*16) for both.
HYB_DPAIRS = 13                  # dense chunk-pairs, one per DMA block
HYB_PPAIRS = 64 - HYB_DPAIRS     # packed pairs (8 per uint16 word group)
HYB_GROUPS = (HYB_PPAIRS + 7) // 8
HYB_DVE_NS = 594                 # est ns per packed pair on DVE
HYB_XFER_D = 1490                # est ns per dense pair DMA
HYB_XFER_G = 1456                # est ns per group / x DMA
HYB_DVE_BIAS = 2400              # est DVE pipeline-head offset (ns)
HYB_WARMUP = 28                  # PE p-state warmup matmuls

def _build(mode: str) -> bass.Bass:
    if mode in _nc_cache:
        return _nc_cache[mode]
    if mode == "hyb":
        nc = _build_hyb()
        _nc_cache[mode] = nc
        return nc
    vdt = VDT[mode]
    g = GROUP[mode]
    nb = KC // g
    f32 = mybir.dt.float32

    nc = bacc.Bacc("TRN2", target_bir_lowering=False, debug=False,
                   num_devices=NCORES)
    xdt = vdt if mode == "f8s" else f32
    x_dram = nc.dram_tensor("x", (128, KC * BC), xdt, kind="ExternalInput")
    v_dram = nc.dram_tensor("v", (nb, 128, g * NPC + 16), vdt,
                            kind="ExternalInput")
    o_dram = nc.dram_tensor("out", (BC, NPC), f32, kind="ExternalOutput")

    vbufs = VBUFS[mode]
    with tile.TileContext(nc) as tc:
        with tc.tile_pool(name="xp", bufs=1) as xp, \
             tc.tile_pool(name="vp", bufs=1) as vp, \
             tc.tile_pool(name="pp", bufs=1, space="PSUM") as pp, \
             tc.tile_pool(name="pt", bufs=1, space="PSUM") as pt, \
             tc.tile_pool(name="gs", bufs=nb) as gate_pool, \
             tc.tile_pool(name="op", bufs=1) as op:

            # --- x preparation: load, relu(+scale), cast/split ---
            x_raw = xp.tile([128, KC * BC], xdt)
            nc.scalar.dma_start(out=x_raw[:], in_=x_dram.ap())

            relu = mybir.ActivationFunctionType.Relu
            if mode == "f8s":
                # host sent fp8e4m3(16*x); relu on DVE keeps the ACT queue
                # free to issue the V-block DMA stream without stalls
                x_use = xp.tile([128, KC * BC], vdt)
                nc.vector.tensor_relu(x_use[:], x_raw[:])
                passes = [x_use]
            elif mode == "f32":
                x_use = xp.tile([128, KC * BC], f32)
                nc.scalar.activation(x_use[:], x_raw[:], relu)
                passes = [x_use]
            elif mode == "f16":
                x_use = xp.tile([128, KC * BC], mybir.dt.float16)
                nc.scalar.activation(x_use[:], x_raw[:], relu)
                passes = [x_use]
            else:  # f8dr: hi/lo split of relu(x)*16
                x_rel = xp.tile([128, KC * BC], f32)
                nc.scalar.activation(x_rel[:], x_raw[:], relu,
                                     scale=X_SCALE[mode])
                x_hi = xp.tile([128, KC * BC], vdt)
                nc.vector.tensor_copy(out=x_hi[:], in_=x_rel[:])
                x_hi32 = xp.tile([128, KC * BC], f32)
                nc.vector.tensor_copy(out=x_hi32[:], in_=x_hi[:])
                resid = xp.tile([128, KC * BC], f32)
                nc.vector.tensor_sub(resid[:], x_rel[:], x_hi32[:])
                x_lo = xp.tile([128, KC * BC], vdt)
                nc.vector.tensor_copy(out=x_lo[:], in_=resid[:])
                passes = [x_hi, x_lo]

            psum = pp.tile([BC, NPC], f32)

            # Walrus allows only ONE sem-wait per DMA instruction, but a
            # v-block DMA into a reused pool slot needs two: WAR on the
            # stale tile's PE readers + WAW on the slot's previous DMA
            # (Tile doesn't collapse waits transitively). Fix:
            #  - every block ends with a tiny "token" matmul into a
            #    dedicated PSUM bank (last PE op touching the block's tile)
            #  - before reusing a slot, ACT copies that token from PSUM
            #    into the stale tile: this gate carries the single PE wait
            #    and its write WAW-orders it before the real DMA on ACT
            #  - the real DMA (also issued from ACT) then carries only the
            #    DMA-lane WAW wait: every instruction has <= 1 sem wait.
            vtiles: list = []
            tok = []
            vts = []
            for j in range(vbufs):
                tok_t = pt.tile([1, 16], f32, tag=f"tok{j}")
                tok.append(tok_t)
                vt_t = vp.tile([128, g * NPC + 16], vdt, tag=f"vt{j}")
                vts.append(vt_t)
            def gate(b):
                if b >= vbufs:
                    stale = vtiles[b - vbufs]
                    # absorb the stale slot's DMA-lane tick into ACT
                    # program order (1 wait: old DMA lane); fresh scratch
                    # slot every time so no WAW self-wait accumulates
                    pg_t = gate_pool.tile([1, 16], f32, tag="pg")
                    nc.scalar.copy(pg_t[:], stale[0:1, 16:32])
                    # carry the PE release (1 wait: PE >= token-mm), and
                    # WAW-order the real DMA behind us on ACT via the junk
                    # pad columns (PE never reads those)
                    nc.scalar.copy(stale[0:1, g * NPC:g * NPC + 16],
                                   tok[(b - vbufs) % vbufs][:])

            def token_mm(b, vt2d, lhs_src):
                nc.tensor.matmul(tok[b % vbufs][:], lhsT=lhs_src[:, 0:1],
                                 rhs=vt2d[:, 0:16], start=True, stop=True)

            # --- main loop: stream V blocks, accumulate matmuls ---
            if mode in ("f8dr", "f8s"):
                dr = mybir.MatmulPerfMode.DoubleRow
                gg_per_block = g // 2
                for b in range(nb):
                    gate(b)
                    vt2d = vts[b % vbufs]
                    vtiles.append(vt2d)
                    nc.scalar.dma_start(out=vt2d[:], in_=v_dram.ap()[b])
                    vt = vt2d[:, 0:g * NPC].rearrange(
                        "p (gg j n) -> p gg j n", gg=gg_per_block, j=2)
                    for gg in range(gg_per_block):
                        cc = b * gg_per_block + gg   # 0..63 double-chunks
                        first = cc == 0
                        last = cc == KC // 2 - 1
                        for n in range(NPC // NT):
                            rhs = vt[:, gg, :, n * NT:(n + 1) * NT]
                            for ip, xpass in enumerate(passes):
                                lhsT = xpass[:, 2 * cc * BC:(2 * cc + 2) * BC]
                                lhsT = lhsT.rearrange(
                                    "p (j m) -> p j m", j=2)
                                nc.tensor.matmul(
                                    psum[:, n * NT:(n + 1) * NT],
                                    lhsT=lhsT, rhs=rhs,
                                    start=(first and ip == 0),
                                    stop=(last and ip == len(passes) - 1),
                                    perf_mode=dr)
                    token_mm(b, vt2d, passes[0])
            else:
                for b in range(nb):
                    gate(b)
                    vt = vts[b % vbufs]
                    vtiles.append(vt)
                    nc.scalar.dma_start(out=vt[:], in_=v_dram.ap()[b])
                    for i in range(g):
                        c = b * g + i
                        lhsT = passes[0][:, c * BC:(c + 1) * BC]
                        for n in range(NPC // NT):
                            nc.tensor.matmul(
                                psum[:, n * NT:(n + 1) * NT],
                                lhsT=lhsT,
                                rhs=vt[:, i * NPC + n * NT:
                                       i * NPC + (n + 1) * NT],
                                start=(c == 0), stop=(c == KC - 1))
                    token_mm(b, vt, passes[0])

            # --- epilogue: flush the last blocks' DMA-lane ticks into ACT
            # so the kernel-tail Drain doesn't exceed its wait capacity ---
            for bb in range(max(0, nb - vbufs), nb):
                fl_t = gate_pool.tile([1, 16], f32, tag="pg")
                nc.scalar.copy(fl_t[:], vtiles[bb][0:1, 16:32])

            # --- epilogue: scale + store ---
            out_t = op.tile([BC, NPC], f32)
            nc.scalar.mul(out_t[:], psum[:], OUT_SCALE[mode])
            nc.scalar.dma_start(out=o_dram.ap(), in_=out_t[:])

    nc.finalize()
    _nc_cache[mode] = nc
    return nc


def _build_hyb() -> bass.Bass:
    f32 = mybir.dt.float32
    f8 = mybir.dt.float8e4
    u16 = mybir.dt.uint16
    alu = mybir.AluOpType
    dr = mybir.MatmulPerfMode.DoubleRow
    nb = HYB_DPAIRS                  # one dense pair per DMA block
    ebufs = 12

    nc = bacc.Bacc("TRN2", target_bir_lowering=False, debug=False,
                   num_devices=NCORES)
    x_dram = nc.dram_tensor("x", (128, KC * BC), f8, kind="ExternalInput")
    v_dram = nc.dram_tensor("v", (nb, 128, 2 * NPC), f8,
                            kind="ExternalInput")
    u_dram = nc.dram_tensor("u", (HYB_GROUPS, 128, NPC), u16,
                            kind="ExternalInput")
    o_dram = nc.dram_tensor("out", (BC, NPC), f32, kind="ExternalOutput")

    # ---- static schedule: estimate producer completion times -------------
    # Every dense pair has its own SBUF tile (no slot reuse -> no gating),
    # so ALL input DMAs issue ungated from the otherwise idle SP queue,
    # groups interleaved early so DVE never starves.
    sp_issue = [("g", 0), ("x",), ("g", 1), ("d", 0), ("g", 2), ("d", 1),
                ("g", 3), ("d", 2), ("g", 4), ("d", 3), ("g", 5), ("d", 4),
                ("g", 6), ("d", 5)] + [("d", b) for b in range(6, nb)]
    sp_issue = [it for it in sp_issue
                if not (it[0] == "g" and it[1] >= HYB_GROUPS)
                and not (it[0] == "d" and it[1] >= nb)]
    t, g_ready, d_ready = 0.0, {}, {}
    for it in sp_issue:
        t += HYB_XFER_D if it[0] == "d" else HYB_XFER_G
        if it[0] == "g":
            g_ready[it[1]] = t
        elif it[0] == "d":
            d_ready[it[1]] = t
    # +3500ns: observed DVE pipeline head (first-op sem chains) — biasing
    # the packed-pair estimates late keeps PE from idling on them (idle
    # resets the PE p-state ramp, halving matmul throughput)
    dve_est, tt = {}, float(HYB_DVE_BIAS)
    for l in range(HYB_PPAIRS):
        tt = max(tt, g_ready[l // 8]) + HYB_DVE_NS
        dve_est[l] = tt
    items = ([("d", b, d_ready[b]) for b in range(nb)]
             + [("v", l, dve_est[l]) for l in range(HYB_PPAIRS)])
    items.sort(key=lambda it: it[2])

    with tile.TileContext(nc) as tc:
        with tc.tile_pool(name="xp", bufs=1) as xp, \
             tc.tile_pool(name="vp", bufs=1) as vp, \
             tc.tile_pool(name="gp", bufs=1) as gp, \
             tc.tile_pool(name="ep", bufs=1) as ep, \
             tc.tile_pool(name="pp", bufs=1, space="PSUM") as pp, \
             tc.tile_pool(name="op", bufs=1) as op:

            gtiles = [gp.tile([128, NPC], u16, tag=f"g{t_}",
                              name=f"gt{t_}") for t_ in range(HYB_GROUPS)]
            x_raw = xp.tile([128, KC * BC], f8)
            x_use = xp.tile([128, KC * BC], f8)
            psum = pp.tile([BC, NPC], f32)
            vts = [vp.tile([128, 2 * NPC], f8, tag=f"vt{b}",
                           name=f"vt{b}") for b in range(nb)]
            etiles = [ep.tile([128, NPC], u16, tag=f"e{j}",
                              name=f"et{j}") for j in range(ebufs)]

            # ---- SP queue: every input DMA, ungated ----------------------
            for it in sp_issue:
                if it[0] == "g":
                    nc.sync.dma_start(out=gtiles[it[1]][:],
                                      in_=u_dram.ap()[it[1]])
                elif it[0] == "x":
                    nc.sync.dma_start(out=x_raw[:], in_=x_dram.ap())
                else:
                    nc.sync.dma_start(out=vts[it[1]][:],
                                      in_=v_dram.ap()[it[1]])

            # ---- PE p-state warmup: ~40 junk matmuls into a spare PSUM
            # bank keep the PE continuously busy from ~0.7us so the ramp
            # (low->mid->full over 3us) completes before real work arrives;
            # every real matmul then runs at the full 0.42ns/row rate ------
            junk = xp.tile([128, 1024], f8, name="junk")
            nc.gpsimd.memset(junk[:], 0)
            jpsum = pp.tile([BC, NT], f32, tag="warm", name="jpsum")
            jlhs = junk[:, 0:2 * BC].rearrange("p (j m) -> p j m", j=2)
            jrhs = junk[:].rearrange("p (j n) -> p j n", j=2)
            for w in range(HYB_WARMUP):
                nc.tensor.matmul(jpsum[:], lhsT=jlhs, rhs=jrhs,
                                 start=True, stop=True, perf_mode=dr)

            # ---- ACT: relu in slices (PE starts after the first) ---------
            relu = mybir.ActivationFunctionType.Relu
            q1 = KC * BC // 4
            nc.scalar.activation(x_use[:, 0:q1], x_raw[:, 0:q1], relu)
            nc.scalar.activation(x_use[:, q1:], x_raw[:, q1:], relu)

            def pair_matmuls(cc, rhs_jn, first, last):
                lhsT = x_use[:, 2 * cc * BC:(2 * cc + 2) * BC].rearrange(
                    "p (j m) -> p j m", j=2)
                for n in range(NPC // NT):
                    nc.tensor.matmul(
                        psum[:, n * NT:(n + 1) * NT], lhsT=lhsT,
                        rhs=rhs_jn[:, :, n * NT:(n + 1) * NT],
                        start=first, stop=last, perf_mode=dr)

            # ---- matmuls (+ inline expansion) in estimated-ready order ---
            ecnt = 0
            for idx, it in enumerate(items):
                first = idx == 0
                last = idx == len(items) - 1
                if it[0] == "d":
                    b = it[1]
                    vt = vts[b][:].rearrange("p (j n) -> p j n", j=2)
                    pair_matmuls(b, vt, first, last)
                else:
                    l = it[1]
                    grp, s = l // 8, l % 8
                    slot = etiles[ecnt % ebufs]
                    ecnt += 1
                    if s <= 6:
                        op1, s2 = alu.logical_shift_left, 6 - s
                    else:
                        op1, s2 = alu.logical_shift_right, 1
                    nc.vector.tensor_scalar(
                        out=slot[:], in0=gtiles[grp][:],
                        scalar1=(1 << s) | (1 << (s + 8)), scalar2=s2,
                        op0=alu.bitwise_and, op1=op1)
                    cc = HYB_DPAIRS + l
                    rhs = slot[:].bitcast(f8).rearrange(
                        "p (n j) -> p j n", j=2)
                    pair_matmuls(cc, rhs, first, last)

            # ---- epilogue: scale halves on DVE+ACT in parallel; each
            # half's store issues from a different queue (SP / ACT) so the
            # two output DMAs overlap and each carries one sem wait -------
            hn = NPC // 2
            out_t = op.tile([BC, NPC], f32)
            nc.vector.tensor_scalar_mul(out_t[:, 0:hn], psum[:, 0:hn],
                                        1.0 / (COLS * 16.0))
            nc.scalar.mul(out_t[:, hn:], psum[:, hn:], 1.0 / (COLS * 16.0))
            # pre-issued from SP: the 1.4us HWDGE/DGE issue overhead burns
            # during the stream; each transfer fires as its mul completes
            nc.sync.dma_start(out=o_dram.ap()[:, 0:hn], in_=out_t[:, 0:hn])
            nc.scalar.dma_start(out=o_dram.ap()[:, hn:], in_=out_t[:, hn:])

    nc.finalize()
    return nc


def _prep_inputs(image: np.ndarray, vote_index: np.ndarray, mode: str):
    if mode == "hyb":
        return _prep_inputs_hyb(image, vote_index)
    np_vdt = mybir.dt.np(VDT[mode])
    g = GROUP[mode]
    nb = KC // g

    # x arranged (128, KC*BC): [p, c*32+m] = image_flat[m, c*128+p] * X_SCALE
    x2 = np.ascontiguousarray(image.reshape(BC, K), dtype=np.float32)
    x_arr = np.ascontiguousarray(
        x2.reshape(BC, KC, 128).transpose(2, 1, 0)).reshape(128, KC * BC)
    if mode == "f8s":
        # pre-relu fp8 cast of 16*x: fp8 rounding preserves sign, so
        # relu(fp8(16x)) == fp8(16*relu(x)) — relu itself stays on device
        x_arr = (x_arr * X_SCALE[mode]).astype(np_vdt)

    # v arranged per core: (nb, 128, g*NPC): [b, p, g'*NPC+j] =
    #   V[(b*g+g')*128 + p, core*NPC + j]
    v2 = vote_index.reshape(K, NTOT)
    if np_vdt != np.float32:
        v2 = v2.astype(np_vdt)  # binary 0/1 -> lossless
    # reshape [b, g', p, core, j] -> transpose to [core, b, p, g', j]
    v5 = v2.reshape(nb, g, 128, NCORES, NPC).transpose(3, 0, 2, 1, 4)
    in_maps = []
    for i in range(NCORES):
        vi = np.zeros((nb, 128, g * NPC + 16), dtype=np_vdt)
        vi[:, :, :g * NPC] = v5[i].reshape(nb, 128, g * NPC)
        in_maps.append({"x": x_arr, "v": vi})
    return in_maps


def _prep_inputs_hyb(image: np.ndarray, vote_index: np.ndarray):
    np_f8 = mybir.dt.np(mybir.dt.float8e4)
    nb = HYB_DPAIRS
    dchunks = 2 * HYB_DPAIRS

    # x arranged (128, KC*BC): [p, c*32+m] = image_flat[m, c*128+p] * s(c)
    # s = 16 for dense chunks (V encoded as 1.0), 8 for packed (V reads 2.0)
    x2 = np.ascontiguousarray(image.reshape(BC, K), dtype=np.float32)
    x_arr = np.ascontiguousarray(
        x2.reshape(BC, KC, 128).transpose(2, 1, 0)).reshape(128, KC * BC)
    scales = np.where(np.arange(KC) < dchunks, 16.0, 8.0).astype(np.float32)
    x_arr = (x_arr.reshape(128, KC, BC) * scales[None, :, None]
             ).reshape(128, KC * BC).astype(np_f8)

    v2 = vote_index.reshape(K, NTOT)
    # dense file: (nb, 128, 2*NPC+16); block b col = j*NPC + n, chunk 2b+j
    vd = v2[:dchunks * 128].reshape(nb, 2, 128, NCORES, NPC)
    vd = vd.transpose(3, 0, 2, 1, 4)  # (core, b, p, j, n)
    # packed file: (groups, 128, NPC) uint16; packed pair l = chunks
    # (dchunks+2l, dchunks+2l+1) -> group l//8, bits (l%8, l%8+8)
    pchunks = K // 128 - dchunks
    vp = v2[dchunks * 128:].reshape(pchunks, 128, NCORES, NPC)
    u_all = np.zeros((HYB_GROUPS, 128, NCORES, NPC), dtype=np.uint16)
    for q in range(pchunks):
        l, j = q // 2, q % 2
        bit = (l % 8) + 8 * j
        u_all[l // 8] |= vp[q].astype(np.uint16) << np.uint16(bit)

    in_maps = []
    for i in range(NCORES):
        vi = np.ascontiguousarray(
            vd[i].reshape(nb, 128, 2 * NPC)).astype(np_f8)
        ui = np.ascontiguousarray(u_all[:, :, i, :])
        in_maps.append({"x": x_arr, "v": vi, "u": ui})
    return in_maps


def _run(image, vote_index, mode=None, **run_kwargs):
    mode = mode or MODE
    nc = _build(mode)
    in_maps = _prep_inputs(np.asarray(image), np.asarray(vote_index), mode)
    res = run_bass_kernel_spmd(nc, in_maps, core_ids=list(range(NCORES)),
                               **run_kwargs)
    out = np.concatenate([r["out"] for r in res.results], axis=1)
    return out.reshape(B, C, H, W).astype(np.float32), res


def kernel(image: np.ndarray, vote_index: np.ndarray) -> np.ndarray:
    out, _ = _run(image, vote_index)
    return out

